# revision 7
# baseline (speedup 1.0000x reference)
"""Per-pixel dynamic-filter 5x5 convolution (KPN-style) on 8 TRN2 NeuronCores.

Math: out[b,h,w] = sum_{di,dj,c} img[b, h+di-2, w+dj-2, c] * filts[b, h, w, (di*5+dj)*3+c]
Shapes: img [4,512,512,3] f32, filts [4,512,512,75] f32 -> out [4,512,512] f32.

Strategy (pure data parallel, no cross-core comms):
  - 8 shards = (batch b in 0..3) x (H half in 0..1); each core owns a
    [256, 512] output slab (2 fused 128-row h-tiles).
  - Host prep (per core): img padded + transposed to [row, c, x] fp16; filts
    transposed to [p, di, dj, ht, c, u] on a 514-wide u-grid where the odd-dj
    planes are pre-shifted by one element so every DVE operand slice starts
    4B-aligned (u-slices are uniformly 513 wide at even offsets). The dj==2
    chunks are stored in fp8e4m3 (measured rel_fro 0.0103 vs the 2e-2 gate),
    cutting HBM bytes; their TTs run in DVE 1x mode.
  - On-chip: one img tile per di [p][ht][c][x:520] (rows DMA'd at partition
    offset di). Filts stream in 25 per-(di,dj) chunks on the SP HWDGE ring;
    img/ident/output ride the ACT engine's separate ring. One DVE
    tensor_tensor per (di,dj) computes all six (ht,c) product planes; the
    TensorEngine accumulates them into one 2-bank fp32 PSUM tile via identity
    matmuls (ht-major order). The last chunk is split per ht so its TT,
    matmuls, fp16 PSUM eviction and output DMA pipeline per h-tile.
  - Dummy-matmul warmup inside the first DMA shadow lifts the PE HAM clock
    throttle before the first real accumulation matmuls arrive.
"""

import sys

sys.path.insert(0, "/opt/trn_rl_repo")

import numpy as np
import ml_dtypes

from concourse import bass, bacc, mybir
from concourse.tile import TileContext
from concourse.bass_utils import run_bass_kernel_spmd

B, H, W, C = 4, 512, 512, 3
K = 5
N_CORES = 8
HSH = H // 2  # 256 rows per shard
N_HT = 2  # h-tiles per shard, fused in the free dim
XT = 520  # img x extent: w_img in [-2, 516) -> x = w_img+2 in [0, 518), pad to 520
UP = 514  # padded filts/product plane width (u-grid)
US = 513  # active u-slice width per TT
FP8_DJ = 2  # dj column stored in fp8e4m3
DJ16 = [0, 1, 3, 4]  # fp16 dj columns, in filts16 dram order
N_WARMUP_MM = 90

_F16 = mybir.dt.float16
_F32 = mybir.dt.float32
_F8 = mybir.dt.float8e4

_NC = None


def build_nc():
    """Build the single-core Bass program (identical on all 8 cores)."""
    nc = bacc.Bacc("TRN2")
    img_d = nc.declare_dram_parameter("img", [HSH + 4, C, XT], _F16, isOutput=False)
    filts_d = nc.declare_dram_parameter(
        "filts", [128, K, len(DJ16), N_HT, C, UP], _F16, isOutput=False
    )
    filts8_d = nc.declare_dram_parameter(
        "filts8", [128, K, N_HT, C, UP], _F8, isOutput=False
    )
    ident_d = nc.declare_dram_parameter("ident", [128, 128], _F16, isOutput=False)
    out_d = nc.declare_dram_parameter("out", [HSH, W], _F16, isOutput=True)

    with TileContext(nc) as tc:
        with (
            tc.tile_pool(name="const", bufs=1) as constp,
            tc.tile_pool(name="imgp", bufs=K) as imgp,
            tc.tile_pool(name="filtp", bufs=8) as filtp,
            tc.tile_pool(name="filt8p", bufs=4) as filt8p,
            tc.tile_pool(name="prodp", bufs=8) as prodp,
            tc.tile_pool(name="outp", bufs=2) as outp,
            tc.tile_pool(name="psump", bufs=1, space="PSUM") as psump,
            tc.tile_pool(name="wpsump", bufs=1, space="PSUM") as wpsump,
        ):
            # img tiles, one per di, all resident: rows at partition offset di.
            # Issued on the ACT ring so the filts stream owns the SP ring.
            img_t = []
            for di in range(K):
                t = imgp.tile([128, N_HT, C, XT], _F16, tag="img", name=f"img{di}")
                eng = nc.sync if di == 0 else nc.scalar
                for ht in range(N_HT):
                    eng.dma_start(
                        out=t[:, ht],
                        in_=img_d[di + 128 * ht : di + 128 * ht + 128, :, :],
                    )
                img_t.append(t)

            id_t = constp.tile([128, 128], _F16, tag="id")
            nc.scalar.dma_start(out=id_t[:], in_=ident_d[:])

            # PE warmup: dummy matmuls in the first DMAs' shadow lift HAM.
            wsrc = constp.tile([128, 512], _F16, tag="wsrc")
            nc.gpsimd.memset(wsrc[:], 0.0)
            wps = wpsump.tile([128, 512], _F32)
            for _ in range(N_WARMUP_MM):
                nc.tensor.matmul(wps[:], wsrc[:, :128], wsrc[:], start=True, stop=True)

            ps = psump.tile([128, N_HT, W], _F32, tag="ps", name="ps")

            def emit_mms(p_t, dj, first, last, only_ht=None):
                s = dj & 1
                for ht in range(N_HT) if only_ht is None else [only_ht]:
                    for c in range(C):
                        nc.tensor.matmul(
                            ps[:, ht, :],
                            id_t[:],
                            p_t[:, ht, c, s : s + W],
                            start=(first and c == 0),
                            stop=(last and c == C - 1),
                        )

            for di in range(K):
                for dj in range(K):
                    last = di == K - 1 and dj == K - 1
                    fp8 = dj == FP8_DJ
                    pool = filt8p if fp8 else filtp
                    ft = pool.tile(
                        [128, N_HT, C, UP],
                        _F8 if fp8 else _F16,
                        tag="ft8" if fp8 else "ft",
                        name=f"ft{di}{dj}",
                    )
                    u0 = dj - (dj & 1)  # even img x-slice start
                    p_t = prodp.tile(
                        [128, N_HT, C, UP], _F16, tag="pt", name=f"pt{di}{dj}"
                    )
                    if not last:
                        if fp8:
                            nc.sync.dma_start(out=ft[:], in_=filts8_d[:, di])
                        else:
                            nc.sync.dma_start(
                                out=ft[:], in_=filts_d[:, di, DJ16.index(dj)]
                            )
                        src = img_t[di][:].rearrange("p t c x -> p (t c) x")
                        nc.vector.tensor_tensor(
                            p_t[:].rearrange("p t c u -> p (t c) u")[:, :, 0:US],
                            src[:, :, u0 : u0 + US],
                            ft[:].rearrange("p t c u -> p (t c) u")[:, :, 0:US],
                            mybir.AluOpType.mult,
                        )
                        emit_mms(p_t, dj, di == 0 and dj == 0, False)
                    else:
                        # Final chunk: stream, multiply, accumulate, evict and
                        # store per h-tile so the tail pipelines.
                        dji = DJ16.index(dj)
                        for ht in range(N_HT):
                            nc.sync.dma_start(
                                out=ft[:, ht], in_=filts_d[:, di, dji, ht]
                            )
                            nc.vector.tensor_tensor(
                                p_t[:, ht, :, 0:US],
                                img_t[di][:, ht, :, u0 : u0 + US],
                                ft[:, ht, :, 0:US],
                                mybir.AluOpType.mult,
                            )
                            emit_mms(p_t, dj, False, True, only_ht=ht)
                            o_t = outp.tile([128, W], _F16, tag="ot", name=f"ot{ht}")
                            nc.scalar.copy(out=o_t[:], in_=ps[:, ht, :])
                            nc.scalar.dma_start(
                                out=out_d[128 * ht : 128 * (ht + 1), :], in_=o_t[:]
                            )

    nc.compile()
    return nc


def get_nc():
    global _NC
    if _NC is None:
        _NC = build_nc()
    return _NC


def prepare_in_maps(img_stack: np.ndarray, filts: np.ndarray):
    """Shard + reformat FULL fp32 inputs into per-core fp16/fp8 input maps."""
    ident = np.eye(128, dtype=np.float16)
    in_maps = []
    for core in range(N_CORES):
        b, hh = divmod(core, 2)
        h0 = hh * HSH
        # img: pad h by 2 each side, w by 2 left / 6 right -> [516, 520, 3]
        padded = np.pad(img_stack[b], ((2, 2), (2, XT - W - 2), (0, 0)))
        shard = padded[h0 : h0 + HSH + 4]  # rows h0-2 .. h0+258
        img_p = np.ascontiguousarray(shard.transpose(0, 2, 1)).astype(np.float16)
        # filts -> [p, di, dj, ht, c, u] on the 514-wide u-grid; odd dj
        # planes sit at u=1..513 so all device slices start 4B-aligned.
        f = filts[b, h0 : h0 + HSH].reshape(N_HT, 128, W, K, K, C)
        base = f.transpose(1, 3, 4, 0, 5, 2).astype(np.float16)  # [p,di,dj,ht,c,w]
        filts_p = np.zeros((128, K, len(DJ16), N_HT, C, UP), dtype=np.float16)
        for dji, dj in enumerate(DJ16):
            q = dj & 1
            filts_p[:, :, dji, :, :, q : q + W] = base[:, :, dj]
        filts8_p = np.zeros(
            (128, K, N_HT, C, UP), dtype=ml_dtypes.float8_e4m3fn
        )
        filts8_p[..., 0:W] = base[:, :, FP8_DJ].astype(ml_dtypes.float8_e4m3fn)
        in_maps.append(
            {"img": img_p, "filts": filts_p, "filts8": filts8_p, "ident": ident}
        )
    return in_maps


def assemble_out(results) -> np.ndarray:
    out = np.empty((B, H, W), dtype=np.float32)
    for core in range(N_CORES):
        b, hh = divmod(core, 2)
        out[b, hh * HSH : (hh + 1) * HSH, :] = results[core]["out"].astype(np.float32)
    return out


def kernel(img_stack: np.ndarray, filts: np.ndarray) -> np.ndarray:
    nc = get_nc()
    in_maps = prepare_in_maps(img_stack, filts)
    res = run_bass_kernel_spmd(nc, in_maps, list(range(N_CORES)))
    return assemble_out(res.results)
